# revision 38
# baseline (speedup 1.0000x reference)
"""L-infinity distance "convolution" kernel for Trainium2 (8 NeuronCores).

Computes out[b, co, h, w] = max_acc |weights[co, acc] - patch[b, h, w, acc]| + bias[co]
where patches are 3x3 replicate-padded windows over x (4, 16, 64, 64),
acc = (c, kh, kw) ordered, accl = 16*9 = 144, cout = 64.

Sharding: 8 cores = 4 batches x 2 row-halves. Each core computes a
[2048 positions, 64 cout] shard. No collectives needed.

ALGORITHM (final): log-sum-exp moves the 144-deep max reduction onto the PE:
  max_k |w_k - p_k| = max( max_k (w_k - p_k), max_k (p_k - w_k) )
  max_k (w_k - p_k) ~= (1/t) ln( sum_k e^{t w_k} * e^{-t p_k} ) - centering
The sum over k is a matmul: M1[pos, co] = sum_k B1[k, pos] * A1[k, co].
Both branches are normalized to the SAME exp sign so one Act instruction
serves both: v1 = p - mn >= 0 and v2 = mx - p >= 0 give
  B1 = e^{-t v1},  B2 = e^{-t v2}   (all in (0, 1], no overflow)
  dist*t = max( ln M1 - t*mn, ln M2 + t*mx ) ;  out = dist + bias - delta

Per 4-tile group (tile = 128 positions), engine-balanced to amortize the
Act engine's ~185ns/instr fixed cost:
  DMA:  one fp16 patch load [128,4,144], one fp16 output store
  DVE:  2 native tensor_reduce (mn, mx), 2 broadcast-subtracts into the
        packed v12 tile, 2 tiny scale ops, branch-max, fused scale+bias
  PE:   12 fp16 transposes of v12 into PSUM (3 chunks x 4 tiles),
        16 bf16 matmuls (4 per tile, quadrant-legal K chunks)
  Act:  ONE 1536-wide Exp reading transposed-v from PSUM and writing the
        bf16 B^T matmul operand (the exp IS the PSUM evacuation), ONE
        512-wide Ln over the group's packed PSUM matmul bank
  Pool: u + s epilogue adds (gpsimd ucode has only Add/Multiply/Memset)

The loop is a 6-stage skewed software pipeline over variable-width jobs
(2,2,4,4,2,2 tiles): narrow jobs at the ends shorten pipeline fill/drain,
wide jobs amortize per-instruction fixed costs; the skew keeps each
engine's in-order queue free of cross-job stalls. Stabilizer reduces use
custom dual-source min/max segmented-scan DVE ops (half the pass length
of a native reduce); the v1/v2 packs use per-tile TensorScalarPtr (4x DVE
mode). A-matrices e^{+-t w^T} (bf16) and the replicated bias are
host-prepped, leaving only 2 setup DMAs.

Precision: t=90, fp16 patches/output (|err| <~ 4e-3 abs), bf16 A/B
(~0.4% -> /t -> 5e-5), LSE centering delta = ln2/(2t). Verified ~2e-3
scale-relative absmax against the fixed seed-0 inputs by test.py.
"""

import math

import numpy as np

B, C, H, W = 4, 16, 64, 64
K = 3
COUT = 64
ACC = C * K * K  # 144
HOUT, WOUT = 64, 64
NPOS = HOUT * WOUT  # 4096
NCORES = 8
HALVES = 2
POS_PER_CORE = NPOS // HALVES  # 2048
P = 128  # partitions
NTILES = POS_PER_CORE // P  # 16
GRP = 4  # tiles per group
NGRP = NTILES // GRP  # 4
# packed v12 layout: [v1 0:144 | gap 144:192 | v2 192:336 | pad 336:384]
# so the three 128-col transpose chunks put matmul K-chunks at legal
# partition bases: T2 has b1[128:144]@0 and b2[0:64]@64, T3 has b2[64:144]@0.
PACKW = 3 * P  # 384
V2OFF = 192

T_SHARP = 90.0
DELTA = math.log(2.0) / (2.0 * T_SHARP)  # empirical LSE centering

CFG = {
    "work_bufs": 4,
    "outp_bufs": 4,
    "small_bufs": 8,
    "pst_bufs": 2,
    "psm_bufs": 4,
}

_TRACE = False

_NC_CACHE = None

_OP_CACHE = None


def _lower_scan(spec, ver):
    """Hand-lowered 3-state FSM for a segmented scan (body-agnostic; the
    SUB_DIM_DONE trigger re-seeds per segment, so a [P, seg, n] input with a
    [P, seg, 0-stride-n] output AP yields one reduced value per segment)."""
    import concourse.dve_spec as ds
    from concourse.dve_spec import Trigger

    n_lanes, n_stages = ds.N_LANES[ver], ds.N_STAGES[ver]
    ds._validate_body(spec, ver)
    spec2 = ds._hoist_stream_invariant_ops(spec)
    scans = ds._collect(spec2.body, ds.Scan)
    latches = ds._collect(spec2.body, ds.Latch)
    assert not latches and spec2.accum is None
    p = ds._build_placement(spec2, scans, n_stages, n_lanes)
    seed_ov, step_ov0 = ds._scan_overrides(scans, p.node_stage)
    assert not step_ov0  # regular scans only (no PageIdx)
    step_ov = {}
    for sc in scans:
        d = p.node_stage[sc]
        step_ov[d] = ds._Stage(sc.op, ds._scan_init(sc), sc.expr)
    body_lvs = ds._body_scan_leaves(spec2)
    consume = (ds.Src0 in body_lvs, ds.Src1 in body_lvs)
    states = [
        ds._State(
            placement=p,
            overrides=seed_ov,
            trigger=ds.COUNT_ONCE,
            repeat=1,
            next=(1, 0, 0),
            write_out=False,
        ),
        ds._State(
            placement=p,
            consume=consume,
            trigger=(Trigger.SRC_TENSOR_DONE, Trigger.SUB_DIM_DONE, Trigger.NONE),
            next=(0, 2, 0),
        ),
        ds._State(
            placement=p,
            consume=consume,
            overrides=step_ov,
            trigger=(Trigger.SRC_TENSOR_DONE, Trigger.SUB_DIM_DONE, Trigger.COUNT),
            next=(0, 2, 1),
            repeat=1,
        ),
    ]
    out = [ds._assemble(s) for s in states]
    for u in out:
        u.validate(ver)
    return out


def _get_ops():
    """Register (once) dual-source min/max segmented-scan DVE ops:
    out[seg] = min(or max) over both sources' elements of segment seg.
    Halves the reduce pass length vs a native single-source reduce."""
    global _OP_CACHE
    if _OP_CACHE is not None:
        return _OP_CACHE
    from concourse.dve_spec import Spec, Src0, Src1, C1, AluOp, scan, minn, maxx
    from concourse.dve_uop import DveOpSpec
    import concourse.dve_ops as dve_ops
    from concourse.dve_ops import DveOp

    def _ref_min(in0, in1, s0, s1, imm2):
        v = np.minimum(in0.astype(np.float32), in1.astype(np.float32))
        return np.minimum.accumulate(v, axis=-1).astype(np.float32)

    def _ref_max(in0, in1, s0, s1, imm2):
        v = np.maximum(in0.astype(np.float32), in1.astype(np.float32))
        return np.maximum.accumulate(v, axis=-1).astype(np.float32)

    ops = []
    for name, aop, pair, ref in (
        ("MIN2_SCAN", AluOp.MIN, minn, _ref_min),
        ("MAX2_SCAN", AluOp.MAX, maxx, _ref_max),
    ):
        spec = Spec(body=scan(aop, pair(Src0, Src1), init=C1), reference=ref)
        if name not in dve_ops._SUB_OPCODE_FOR_NAME:
            row = max(dve_ops._SUB_OPCODE_FOR_NAME.values()) + 1
            assert row < 0x20
            dve_ops._SUB_OPCODE_FOR_NAME[name] = row
        row = dve_ops._SUB_OPCODE_FOR_NAME[name]
        shas = {}
        for ver in ("v3", "v4"):
            s = DveOpSpec(name=name, opcode=row, uops=_lower_scan(spec, ver), rd1_en=True)
            dve_ops._COMPILE_CACHE[(name, ver)] = s
            shas[ver] = s.sha(ver)
        op = DveOp(name, spec, subdim=True, uops_sha=shas)
        if all(o.name != name for o in dve_ops.OPS):
            dve_ops.OPS.append(op)
            dve_ops.CUSTOM_DVE_SPECS[name] = spec
        ops.append(op)
    _OP_CACHE = tuple(ops)
    return _OP_CACHE


def _patch_act_tables():
    """Make Exp and Ln resolve only to the combined exp+ln table set so the
    act-table inserter emits one LoadActFuncSet instead of thrashing between
    the exp-only and ln-only sets (1283ns per swap)."""
    import concourse.bacc as bacc
    import concourse.mybir as mybir
    from concourse.hw_specs import get_activation_tables as _orig

    if getattr(bacc, "_act_tables_patched", False):
        return
    AF = mybir.ActivationFunctionType

    def _patched(arch):
        t = {k: set(v) for k, v in _orig(arch).items()}
        both = [k for k, v in t.items() if AF.Exp in v and AF.Ln in v]
        if both:
            keep = both[0]
            for k in t:
                if k != keep:
                    t[k] -= {AF.Exp, AF.Ln}
        return t

    bacc.get_activation_tables = _patched
    bacc._act_tables_patched = True


def _build_bass():
    import concourse.bass as bass
    import concourse.bacc as bacc
    import concourse.mybir as mybir
    import concourse.tile as tile
    from concourse.alu_op_type import AluOpType
    from concourse.masks import make_identity

    _patch_act_tables()
    op_min2, op_max2 = _get_ops()
    AF = mybir.ActivationFunctionType
    f32, f16, bf16 = mybir.dt.float32, mybir.dt.float16, mybir.dt.bfloat16

    def bcast(ap, dims):
        """Raw AP with the partition dim of `ap` plus explicit free dims."""
        return bass.AP(tensor=ap.tensor, offset=ap.offset, ap=[ap.ap[0]] + dims)

    nc = bacc.Bacc("TRN2", target_bir_lowering=False, debug=False, num_devices=NCORES)
    patches_d = nc.dram_tensor(
        "patches", [POS_PER_CORE, ACC], f16, kind="ExternalInput"
    )
    # host-prepped: packed bf16 A-matrices e^{+-t w^T} and fp32 bias-DELTA
    # replicated 4x along cout (see _host_prep for the column layout)
    wpack_d = nc.dram_tensor("wpack", [P, 4 * COUT], bf16, kind="ExternalInput")
    brep_d = nc.dram_tensor("brep", [P, GRP * COUT], f32, kind="ExternalInput")
    out_d = nc.dram_tensor("out", [POS_PER_CORE, COUT], f16, kind="ExternalOutput")

    with tile.TileContext(nc) as tc:
        with (
            tc.tile_pool(name="consts", bufs=1) as consts,
            tc.tile_pool(name="work", bufs=CFG["work_bufs"]) as work,
            tc.tile_pool(name="outp", bufs=CFG["outp_bufs"]) as outp,
            tc.tile_pool(name="small", bufs=CFG["small_bufs"]) as small,
            tc.tile_pool(name="pst", bufs=CFG["pst_bufs"], space="PSUM") as pst,
            tc.tile_pool(name="psm", bufs=CFG["psm_bufs"], space="PSUM") as psm,
        ):
            # ---- setup constants (DMAs issued after the first patch DMA so
            # HWDGE serves group 0's input first) ----
            ident = consts.tile([P, P], f16)
            make_identity(nc, ident)
            wpack = consts.tile([P, 4 * COUT], bf16)
            brep = consts.tile([P, GRP * COUT], f32)
            a1hi = wpack[:, 0:COUT]  # e^{+t wT[0:128]}
            a2hi = wpack[64:P, COUT : 2 * COUT]  # e^{-t wT[0:64]} @ parts 64:128
            a2lo = wpack[0:80, 2 * COUT : 3 * COUT]  # e^{-t wT[64:144]} @ 0:80
            a1lo = wpack[0:16, 3 * COUT : 4 * COUT]  # e^{+t wT[128:144]} @ 0:16

            # ---- 6-stage skewed software pipeline over variable-size jobs.
            # Narrow (2-tile) jobs at the head shorten the fill (first exp
            # starts sooner) and at the tail shorten the serial drain chain;
            # 4-tile jobs in the middle amortize per-instruction overhead.
            # S0: DMA in + DVE stabilizers/v-pack + PE transposes
            # S1: Act exp (PSUM->SBUF B^T)   S2: PE matmuls   S3: Act ln
            # S4: Pool u+s adds              S5: DVE max + scale/bias + DMA out
            JOBS = [(0, 2), (2, 2), (4, 4), (8, 4), (12, 2), (14, 2)]
            st = {}

            def S0(jb):
                t0, nt = JOBS[jb]
                ptg = work.tile([P, GRP, ACC], f16, tag="ptg")
                nc.sync.dma_start(
                    out=ptg[:, 0:nt, :],
                    in_=patches_d[t0 * P : (t0 + nt) * P, :].rearrange(
                        "(t p) a -> p t a", p=P
                    ),
                )
                # per-(position, tile) stabilizers (fp32 so the per-tile
                # tensor_scalar 4x fast path can consume them directly)
                r1 = small.tile([P, GRP], f32, tag="r1")  # mn
                r2 = small.tile([P, GRP], f32, tag="r2")  # mx
                v12 = work.tile([P, GRP, PACKW], f16, tag="v12")
                if jb < CFG["work_bufs"]:
                    # zero all gap/pad cols once per rotating buffer; v1/v2
                    # writes never touch them so later rounds inherit zeros
                    nc.gpsimd.memset(v12[:, :, ACC:V2OFF], 0.0)
                    nc.gpsimd.memset(v12[:, :, V2OFF + ACC : PACKW], 0.0)
                HA = ACC // 2  # 72
                for dst, op, init in ((r1, op_min2, 3.0e38), (r2, op_max2, -3.0e38)):
                    d0 = dst[:, :]
                    squash = bass.AP(
                        tensor=d0.tensor, offset=d0.offset,
                        ap=[d0.ap[0], [1, nt], [0, HA]],
                    )
                    nc.vector._custom_dve(
                        op, out=squash,
                        in0=ptg[:, 0:nt, 0:HA], in1=ptg[:, 0:nt, HA:ACC],
                        s0=0.0, s1=init,
                    )
                # v1 = p - mn (>=0), per-tile TensorScalar (4x DVE mode)
                for i in range(nt):
                    nc.vector.tensor_scalar(
                        out=v12[:, i, 0:ACC], in0=ptg[:, i, :],
                        scalar1=r1[:, i : i + 1], scalar2=None,
                        op0=AluOpType.subtract,
                    )
                # v2 = (p - mx)*(-1) = mx - p (>=0)
                for i in range(nt):
                    nc.vector.tensor_scalar(
                        out=v12[:, i, V2OFF : V2OFF + ACC], in0=ptg[:, i, :],
                        scalar1=r2[:, i : i + 1], scalar2=-1.0,
                        op0=AluOpType.subtract, op1=AluOpType.mult,
                    )
                # epilogue scalars (negated so the epilogue uses Pool-legal
                # adds): s1 = -t*mn, s2 = +t*mx
                s12 = small.tile([P, 2 * GRP], f32, tag="s12")
                nc.vector.tensor_scalar(
                    out=s12[:, 0:nt], in0=r1[:, 0:nt], scalar1=-T_SHARP,
                    scalar2=None, op0=AluOpType.mult,
                )
                nc.vector.tensor_scalar(
                    out=s12[:, GRP : GRP + nt], in0=r2[:, 0:nt], scalar1=T_SHARP,
                    scalar2=None, op0=AluOpType.mult,
                )
                # transpose v12 on the PE: psumT[:, i, k, :] = chunk k of tile i
                # with partitions = acc-chunk, free = position; chunk-0s first
                # (they only need v1)
                psumT = pst.tile([P, GRP, 3, P], f16, tag="pT")
                for k in range(3):
                    for i in range(nt):
                        nc.tensor.transpose(
                            psumT[:, i, k, :],
                            v12[:, i, k * P : (k + 1) * P],
                            ident[:, :],
                        )
                st[jb] = {"psumT": psumT, "s12": s12}

            def S1(jb):
                # ONE exp for the whole job: B^T = e^{-t * vT}, bf16, and it
                # doubles as the PSUM->SBUF evacuation
                t0, nt = JOBS[jb]
                psumT = st[jb].pop("psumT")
                bT = work.tile([P, GRP, 3, P], bf16, tag="bT")
                nc.scalar.activation(
                    bT[:, 0:nt, :, :], psumT[:, 0:nt, :, :], AF.Exp, scale=-T_SHARP
                )
                st[jb]["bT"] = bT

            def S2(jb):
                # matmuls: m12 layout [tile, branch, co] so a job's valid
                # region is contiguous
                t0, nt = JOBS[jb]
                bT = st[jb].pop("bT")
                m12 = psm.tile([P, 2 * GRP * COUT], f32, tag="m12")
                m12v = m12.rearrange("p (i j c) -> p i j c", i=GRP, j=2)
                for i in range(nt):
                    nc.tensor.matmul(
                        m12v[:, i, 0, :], bT[:, i, 0, :], a1hi,
                        start=True, stop=False,
                    )
                    nc.tensor.matmul(
                        m12v[:, i, 0, :], bT[0:16, i, 1, :], a1lo,
                        start=False, stop=True,
                    )
                    nc.tensor.matmul(
                        m12v[:, i, 1, :], bT[64:P, i, 1, :], a2hi,
                        start=True, stop=False,
                    )
                    nc.tensor.matmul(
                        m12v[:, i, 1, :], bT[0:80, i, 2, :], a2lo,
                        start=False, stop=True,
                    )
                st[jb]["m12"] = m12

            def S3(jb):
                t0, nt = JOBS[jb]
                m12 = st[jb].pop("m12")
                u12 = outp.tile([P, 2 * GRP * COUT], f32, tag="u12")
                nc.scalar.activation(
                    u12[:, 0 : nt * 2 * COUT], m12[:, 0 : nt * 2 * COUT], AF.Ln
                )
                st[jb]["u12"] = u12

            def S4(jb):
                t0, nt = JOBS[jb]
                u12 = st[jb].pop("u12")
                s12 = st[jb].pop("s12")
                # e = u + s (broadcast per branch/tile); u12/e12 are
                # [tile, branch, co] so each branch is a strided 3D view
                e12 = outp.tile([P, GRP, 2, COUT], f32, tag="e12")
                for j in range(2):
                    nc.gpsimd.tensor_tensor(
                        out=e12[:, 0:nt, j, :],
                        in0=u12[:, 0 : nt * 2 * COUT].rearrange(
                            "p (i j c) -> p i j c", i=nt, j=2
                        )[:, :, j, :],
                        in1=bcast(
                            s12[:, j * GRP : j * GRP + nt], [[1, nt], [0, COUT]]
                        ),
                        op=AluOpType.add,
                    )
                st[jb]["e12"] = e12

            def S5(jb):
                t0, nt = JOBS[jb]
                e12 = st.pop(jb)["e12"]
                d0 = outp.tile([P, GRP * COUT], f32, tag="d0")
                d0v = d0.rearrange("p (i c) -> p i c", i=GRP)
                nc.vector.tensor_tensor(
                    out=d0v[:, 0:nt, :],
                    in0=e12[:, 0:nt, 0, :],
                    in1=e12[:, 0:nt, 1, :],
                    op=AluOpType.max,
                )
                dg = outp.tile([P, GRP, COUT], f16, tag="dg")
                nc.vector.scalar_tensor_tensor(
                    out=dg[:, 0:nt, :].rearrange("p i c -> p (i c)"),
                    in0=d0[:, 0 : nt * COUT],
                    scalar=1.0 / T_SHARP,
                    in1=brep[:, 0 : nt * COUT],
                    op0=AluOpType.mult,
                    op1=AluOpType.add,
                )
                nc.sync.dma_start(
                    out=out_d[t0 * P : (t0 + nt) * P, :].rearrange(
                        "(t p) a -> p t a", p=P
                    ),
                    in_=dg[:, 0:nt, :],
                )

            stages = [(5, S5), (4, S4), (3, S3), (2, S2), (1, S1), (0, S0)]
            NJOBS = len(JOBS)
            for c in range(NJOBS + 5):
                for s, fn in stages:
                    jb = c - s
                    if 0 <= jb < NJOBS:
                        fn(jb)
                if c == 1:
                    # const DMAs behind jobs 0+1's patch DMAs in the HWDGE
                    # queue (their arrival gates the DVE stream start); first
                    # const consumers (S2 matmuls / S5 bias) are clocks away
                    nc.sync.dma_start(out=wpack[:, :], in_=wpack_d[:, :])
                    nc.sync.dma_start(out=brep[:, :], in_=brep_d[:, :])
    nc.compile()
    return nc


def _host_prep(inputs):
    import ml_dtypes

    x = np.asarray(inputs["x"], dtype=np.float32)
    weights = np.asarray(inputs["weights"], dtype=np.float32)
    bias = np.asarray(inputs["bias"], dtype=np.float32)
    assert x.shape == (B, C, H, W)
    assert weights.shape == (COUT, ACC)

    x_pad = np.pad(x, ((0, 0), (0, 0), (1, 1), (1, 1)), mode="edge")
    from numpy.lib.stride_tricks import sliding_window_view

    pw = sliding_window_view(x_pad, (K, K), axis=(2, 3))  # (B, C, HOUT, WOUT, K, K)
    patches = (
        np.ascontiguousarray(pw.transpose(0, 2, 3, 1, 4, 5))
        .reshape(B, NPOS, ACC)
        .astype(np.float16)
    )
    wt = weights.T.astype(np.float64)  # [144, 64]
    # packed bf16 A-matrices, partition-placed for the matmul K-chunk bases
    wpack = np.zeros((P, 4 * COUT), dtype=np.float64)
    wpack[0:P, 0:COUT] = np.exp(T_SHARP * wt[0:P])  # a1hi
    wpack[64:P, COUT : 2 * COUT] = np.exp(-T_SHARP * wt[0:64])  # a2hi
    wpack[0:80, 2 * COUT : 3 * COUT] = np.exp(-T_SHARP * wt[64:ACC])  # a2lo
    wpack[0:16, 3 * COUT : 4 * COUT] = np.exp(T_SHARP * wt[P:ACC])  # a1lo
    wpack = wpack.astype(ml_dtypes.bfloat16)
    brep = np.tile(bias.reshape(1, COUT) - DELTA, (P, GRP)).astype(np.float32)
    return patches, wpack, brep


def _get_nc():
    global _NC_CACHE
    if _NC_CACHE is None:
        _NC_CACHE = _build_bass()
    return _NC_CACHE


def _run(inputs, trace=False):
    from concourse.bass_utils import run_bass_kernel_spmd

    patches, wpack, brep = _host_prep(inputs)
    in_maps = []
    for core in range(NCORES):
        b, half = core // HALVES, core % HALVES
        shard = np.ascontiguousarray(
            patches[b, half * POS_PER_CORE : (half + 1) * POS_PER_CORE, :]
        )
        in_maps.append({"patches": shard, "wpack": wpack, "brep": brep})

    nc = _get_nc()
    res = run_bass_kernel_spmd(nc, in_maps, core_ids=list(range(NCORES)), trace=trace)

    rows_per_half = POS_PER_CORE // WOUT  # 32
    out = np.empty((B, COUT, HOUT, WOUT), dtype=np.float32)
    for core in range(NCORES):
        b, half = core // HALVES, core % HALVES
        o = res.results[core]["out"].astype(np.float32)  # [POS_PER_CORE, COUT]
        out[b, :, half * rows_per_half : (half + 1) * rows_per_half, :] = o.T.reshape(
            COUT, rows_per_half, WOUT
        )
    return out, res


def kernel(**inputs) -> np.ndarray:
    out, _ = _run(inputs, trace=_TRACE)
    return out


# revision 39
# speedup vs baseline: 1.0066x; 1.0066x over previous
"""L-infinity distance "convolution" kernel for Trainium2 (8 NeuronCores).

Computes out[b, co, h, w] = max_acc |weights[co, acc] - patch[b, h, w, acc]| + bias[co]
where patches are 3x3 replicate-padded windows over x (4, 16, 64, 64),
acc = (c, kh, kw) ordered, accl = 16*9 = 144, cout = 64.

Sharding: 8 cores = 4 batches x 2 row-halves. Each core computes a
[2048 positions, 64 cout] shard. No collectives needed.

ALGORITHM (final): log-sum-exp moves the 144-deep max reduction onto the PE:
  max_k |w_k - p_k| = max( max_k (w_k - p_k), max_k (p_k - w_k) )
  max_k (w_k - p_k) ~= (1/t) ln( sum_k e^{t w_k} * e^{-t p_k} ) - centering
The sum over k is a matmul: M1[pos, co] = sum_k B1[k, pos] * A1[k, co].
Both branches are normalized to the SAME exp sign so one Act instruction
serves both: v1 = p - mn >= 0 and v2 = mx - p >= 0 give
  B1 = e^{-t v1},  B2 = e^{-t v2}   (all in (0, 1], no overflow)
  dist*t = max( ln M1 - t*mn, ln M2 + t*mx ) ;  out = dist + bias - delta

Per 4-tile group (tile = 128 positions), engine-balanced to amortize the
Act engine's ~185ns/instr fixed cost:
  DMA:  one fp16 patch load [128,4,144], one fp16 output store
  DVE:  2 native tensor_reduce (mn, mx), 2 broadcast-subtracts into the
        packed v12 tile, 2 tiny scale ops, branch-max, fused scale+bias
  PE:   12 fp16 transposes of v12 into PSUM (3 chunks x 4 tiles),
        16 bf16 matmuls (4 per tile, quadrant-legal K chunks)
  Act:  ONE 1536-wide Exp reading transposed-v from PSUM and writing the
        bf16 B^T matmul operand (the exp IS the PSUM evacuation), ONE
        512-wide Ln over the group's packed PSUM matmul bank
  Pool: u + s epilogue adds (gpsimd ucode has only Add/Multiply/Memset)

The loop is a 6-stage skewed software pipeline over variable-width jobs
(2,2,4,4,2,2 tiles): narrow jobs at the ends shorten pipeline fill/drain,
wide jobs amortize per-instruction fixed costs; the skew keeps each
engine's in-order queue free of cross-job stalls. Stabilizer reduces use
custom dual-source min/max segmented-scan DVE ops (half the pass length
of a native reduce); the v1/v2 packs use per-tile TensorScalarPtr (4x DVE
mode). A-matrices e^{+-t w^T} (bf16) and the replicated bias are
host-prepped, leaving only 2 setup DMAs.

Precision: t=90, fp16 patches/output (|err| <~ 4e-3 abs), bf16 A/B
(~0.4% -> /t -> 5e-5), LSE centering delta = ln2/(2t). Verified ~2e-3
scale-relative absmax against the fixed seed-0 inputs by test.py.
"""

import math

import numpy as np

B, C, H, W = 4, 16, 64, 64
K = 3
COUT = 64
ACC = C * K * K  # 144
HOUT, WOUT = 64, 64
NPOS = HOUT * WOUT  # 4096
NCORES = 8
HALVES = 2
POS_PER_CORE = NPOS // HALVES  # 2048
P = 128  # partitions
NTILES = POS_PER_CORE // P  # 16
GRP = 4  # tiles per group
NGRP = NTILES // GRP  # 4
# packed v12 layout: [v1 0:144 | gap 144:192 | v2 192:336 | pad 336:384]
# so the three 128-col transpose chunks put matmul K-chunks at legal
# partition bases: T2 has b1[128:144]@0 and b2[0:64]@64, T3 has b2[64:144]@0.
PACKW = 3 * P  # 384
V2OFF = 192

T_SHARP = 90.0
DELTA = math.log(2.0) / (2.0 * T_SHARP)  # empirical LSE centering

CFG = {
    "work_bufs": 4,
    "outp_bufs": 4,
    "small_bufs": 8,
    "pst_bufs": 2,
    "psm_bufs": 4,
}

_TRACE = False

_NC_CACHE = None

_OP_CACHE = None


def _lower_scan(spec, ver):
    """Hand-lowered 3-state FSM for a segmented scan (body-agnostic; the
    SUB_DIM_DONE trigger re-seeds per segment, so a [P, seg, n] input with a
    [P, seg, 0-stride-n] output AP yields one reduced value per segment)."""
    import concourse.dve_spec as ds
    from concourse.dve_spec import Trigger

    n_lanes, n_stages = ds.N_LANES[ver], ds.N_STAGES[ver]
    ds._validate_body(spec, ver)
    spec2 = ds._hoist_stream_invariant_ops(spec)
    scans = ds._collect(spec2.body, ds.Scan)
    latches = ds._collect(spec2.body, ds.Latch)
    assert not latches and spec2.accum is None
    p = ds._build_placement(spec2, scans, n_stages, n_lanes)
    seed_ov, step_ov0 = ds._scan_overrides(scans, p.node_stage)
    assert not step_ov0  # regular scans only (no PageIdx)
    step_ov = {}
    for sc in scans:
        d = p.node_stage[sc]
        step_ov[d] = ds._Stage(sc.op, ds._scan_init(sc), sc.expr)
    body_lvs = ds._body_scan_leaves(spec2)
    consume = (ds.Src0 in body_lvs, ds.Src1 in body_lvs)
    states = [
        ds._State(
            placement=p,
            overrides=seed_ov,
            trigger=ds.COUNT_ONCE,
            repeat=1,
            next=(1, 0, 0),
            write_out=False,
        ),
        ds._State(
            placement=p,
            consume=consume,
            trigger=(Trigger.SRC_TENSOR_DONE, Trigger.SUB_DIM_DONE, Trigger.NONE),
            next=(0, 2, 0),
        ),
        ds._State(
            placement=p,
            consume=consume,
            overrides=step_ov,
            trigger=(Trigger.SRC_TENSOR_DONE, Trigger.SUB_DIM_DONE, Trigger.COUNT),
            next=(0, 2, 1),
            repeat=1,
        ),
    ]
    out = [ds._assemble(s) for s in states]
    for u in out:
        u.validate(ver)
    return out


def _get_ops():
    """Register (once) dual-source min/max segmented-scan DVE ops:
    out[seg] = min(or max) over both sources' elements of segment seg.
    Halves the reduce pass length vs a native single-source reduce."""
    global _OP_CACHE
    if _OP_CACHE is not None:
        return _OP_CACHE
    from concourse.dve_spec import Spec, Src0, Src1, C1, AluOp, scan, minn, maxx
    from concourse.dve_uop import DveOpSpec
    import concourse.dve_ops as dve_ops
    from concourse.dve_ops import DveOp

    def _ref_min(in0, in1, s0, s1, imm2):
        v = np.minimum(in0.astype(np.float32), in1.astype(np.float32))
        return np.minimum.accumulate(v, axis=-1).astype(np.float32)

    def _ref_max(in0, in1, s0, s1, imm2):
        v = np.maximum(in0.astype(np.float32), in1.astype(np.float32))
        return np.maximum.accumulate(v, axis=-1).astype(np.float32)

    ops = []
    for name, aop, pair, ref in (
        ("MIN2_SCAN", AluOp.MIN, minn, _ref_min),
        ("MAX2_SCAN", AluOp.MAX, maxx, _ref_max),
    ):
        spec = Spec(body=scan(aop, pair(Src0, Src1), init=C1), reference=ref)
        if name not in dve_ops._SUB_OPCODE_FOR_NAME:
            row = max(dve_ops._SUB_OPCODE_FOR_NAME.values()) + 1
            assert row < 0x20
            dve_ops._SUB_OPCODE_FOR_NAME[name] = row
        row = dve_ops._SUB_OPCODE_FOR_NAME[name]
        shas = {}
        for ver in ("v3", "v4"):
            s = DveOpSpec(name=name, opcode=row, uops=_lower_scan(spec, ver), rd1_en=True)
            dve_ops._COMPILE_CACHE[(name, ver)] = s
            shas[ver] = s.sha(ver)
        op = DveOp(name, spec, subdim=True, uops_sha=shas)
        if all(o.name != name for o in dve_ops.OPS):
            dve_ops.OPS.append(op)
            dve_ops.CUSTOM_DVE_SPECS[name] = spec
        ops.append(op)
    _OP_CACHE = tuple(ops)
    return _OP_CACHE


def _patch_act_tables():
    """Make Exp and Ln resolve only to the combined exp+ln table set so the
    act-table inserter emits one LoadActFuncSet instead of thrashing between
    the exp-only and ln-only sets (1283ns per swap)."""
    import concourse.bacc as bacc
    import concourse.mybir as mybir
    from concourse.hw_specs import get_activation_tables as _orig

    if getattr(bacc, "_act_tables_patched", False):
        return
    AF = mybir.ActivationFunctionType

    def _patched(arch):
        t = {k: set(v) for k, v in _orig(arch).items()}
        both = [k for k, v in t.items() if AF.Exp in v and AF.Ln in v]
        if both:
            keep = both[0]
            for k in t:
                if k != keep:
                    t[k] -= {AF.Exp, AF.Ln}
        return t

    bacc.get_activation_tables = _patched
    bacc._act_tables_patched = True


def _build_bass():
    import concourse.bass as bass
    import concourse.bacc as bacc
    import concourse.mybir as mybir
    import concourse.tile as tile
    from concourse.alu_op_type import AluOpType
    from concourse.masks import make_identity

    _patch_act_tables()
    op_min2, op_max2 = _get_ops()
    AF = mybir.ActivationFunctionType
    f32, f16, bf16 = mybir.dt.float32, mybir.dt.float16, mybir.dt.bfloat16

    def bcast(ap, dims):
        """Raw AP with the partition dim of `ap` plus explicit free dims."""
        return bass.AP(tensor=ap.tensor, offset=ap.offset, ap=[ap.ap[0]] + dims)

    nc = bacc.Bacc("TRN2", target_bir_lowering=False, debug=False, num_devices=NCORES)
    patches_d = nc.dram_tensor(
        "patches", [POS_PER_CORE, ACC], f16, kind="ExternalInput"
    )
    # host-prepped: packed bf16 A-matrices e^{+-t w^T} and fp32 bias-DELTA
    # replicated 4x along cout (see _host_prep for the column layout)
    wpack_d = nc.dram_tensor("wpack", [P, 4 * COUT], bf16, kind="ExternalInput")
    brep_d = nc.dram_tensor("brep", [P, GRP * COUT], f16, kind="ExternalInput")
    out_d = nc.dram_tensor("out", [POS_PER_CORE, COUT], f16, kind="ExternalOutput")

    with tile.TileContext(nc) as tc:
        with (
            tc.tile_pool(name="consts", bufs=1) as consts,
            tc.tile_pool(name="work", bufs=CFG["work_bufs"]) as work,
            tc.tile_pool(name="outp", bufs=CFG["outp_bufs"]) as outp,
            tc.tile_pool(name="small", bufs=CFG["small_bufs"]) as small,
            tc.tile_pool(name="pst", bufs=CFG["pst_bufs"], space="PSUM") as pst,
            tc.tile_pool(name="psm", bufs=CFG["psm_bufs"], space="PSUM") as psm,
        ):
            # ---- setup constants (DMAs issued after the first patch DMA so
            # HWDGE serves group 0's input first) ----
            ident = consts.tile([P, P], f16)
            make_identity(nc, ident)
            wpack = consts.tile([P, 4 * COUT], bf16)
            brep = consts.tile([P, GRP * COUT], f16)
            a1hi = wpack[:, 0:COUT]  # e^{+t wT[0:128]}
            a2hi = wpack[64:P, COUT : 2 * COUT]  # e^{-t wT[0:64]} @ parts 64:128
            a2lo = wpack[0:80, 2 * COUT : 3 * COUT]  # e^{-t wT[64:144]} @ 0:80
            a1lo = wpack[0:16, 3 * COUT : 4 * COUT]  # e^{+t wT[128:144]} @ 0:16

            # ---- 6-stage skewed software pipeline over variable-size jobs.
            # Narrow (2-tile) jobs at the head shorten the fill (first exp
            # starts sooner) and at the tail shorten the serial drain chain;
            # 4-tile jobs in the middle amortize per-instruction overhead.
            # S0: DMA in + DVE stabilizers/v-pack + PE transposes
            # S1: Act exp (PSUM->SBUF B^T)   S2: PE matmuls   S3: Act ln
            # S4: Pool u+s adds              S5: DVE max + scale/bias + DMA out
            JOBS = [(0, 2), (2, 2), (4, 4), (8, 4), (12, 2), (14, 2)]
            st = {}

            def S0(jb):
                t0, nt = JOBS[jb]
                ptg = work.tile([P, GRP, ACC], f16, tag="ptg")
                nc.sync.dma_start(
                    out=ptg[:, 0:nt, :],
                    in_=patches_d[t0 * P : (t0 + nt) * P, :].rearrange(
                        "(t p) a -> p t a", p=P
                    ),
                )
                # per-(position, tile) stabilizers (fp32 so the per-tile
                # tensor_scalar 4x fast path can consume them directly)
                r1 = small.tile([P, GRP], f32, tag="r1")  # mn
                r2 = small.tile([P, GRP], f32, tag="r2")  # mx
                v12 = work.tile([P, GRP, PACKW], f16, tag="v12")
                if jb < CFG["work_bufs"]:
                    # zero all gap/pad cols once per rotating buffer; v1/v2
                    # writes never touch them so later rounds inherit zeros
                    nc.gpsimd.memset(v12[:, :, ACC:V2OFF], 0.0)
                    nc.gpsimd.memset(v12[:, :, V2OFF + ACC : PACKW], 0.0)
                HA = ACC // 2  # 72
                for dst, op, init in ((r1, op_min2, 3.0e38), (r2, op_max2, -3.0e38)):
                    d0 = dst[:, :]
                    squash = bass.AP(
                        tensor=d0.tensor, offset=d0.offset,
                        ap=[d0.ap[0], [1, nt], [0, HA]],
                    )
                    nc.vector._custom_dve(
                        op, out=squash,
                        in0=ptg[:, 0:nt, 0:HA], in1=ptg[:, 0:nt, HA:ACC],
                        s0=0.0, s1=init,
                    )
                # v1 = p - mn (>=0), per-tile TensorScalar (4x DVE mode)
                for i in range(nt):
                    nc.vector.tensor_scalar(
                        out=v12[:, i, 0:ACC], in0=ptg[:, i, :],
                        scalar1=r1[:, i : i + 1], scalar2=None,
                        op0=AluOpType.subtract,
                    )
                # v2 = (p - mx)*(-1) = mx - p (>=0)
                for i in range(nt):
                    nc.vector.tensor_scalar(
                        out=v12[:, i, V2OFF : V2OFF + ACC], in0=ptg[:, i, :],
                        scalar1=r2[:, i : i + 1], scalar2=-1.0,
                        op0=AluOpType.subtract, op1=AluOpType.mult,
                    )
                # epilogue scalars (negated so the epilogue uses Pool-legal
                # adds): s1 = -t*mn, s2 = +t*mx
                s12 = small.tile([P, 2 * GRP], f32, tag="s12")
                nc.vector.tensor_scalar(
                    out=s12[:, 0:nt], in0=r1[:, 0:nt], scalar1=-T_SHARP,
                    scalar2=None, op0=AluOpType.mult,
                )
                nc.vector.tensor_scalar(
                    out=s12[:, GRP : GRP + nt], in0=r2[:, 0:nt], scalar1=T_SHARP,
                    scalar2=None, op0=AluOpType.mult,
                )
                # transpose v12 on the PE: psumT[:, i, k, :] = chunk k of tile i
                # with partitions = acc-chunk, free = position; chunk-0s first
                # (they only need v1)
                psumT = pst.tile([P, GRP, 3, P], f16, tag="pT")
                for k in range(3):
                    for i in range(nt):
                        nc.tensor.transpose(
                            psumT[:, i, k, :],
                            v12[:, i, k * P : (k + 1) * P],
                            ident[:, :],
                        )
                st[jb] = {"psumT": psumT, "s12": s12}

            def S1(jb):
                # ONE exp for the whole job: B^T = e^{-t * vT}, bf16, and it
                # doubles as the PSUM->SBUF evacuation
                t0, nt = JOBS[jb]
                psumT = st[jb].pop("psumT")
                bT = work.tile([P, GRP, 3, P], bf16, tag="bT")
                nc.scalar.activation(
                    bT[:, 0:nt, :, :], psumT[:, 0:nt, :, :], AF.Exp, scale=-T_SHARP
                )
                st[jb]["bT"] = bT

            def S2(jb):
                # matmuls: m12 layout [tile, branch, co] so a job's valid
                # region is contiguous
                t0, nt = JOBS[jb]
                bT = st[jb].pop("bT")
                m12 = psm.tile([P, 2 * GRP * COUT], f32, tag="m12")
                m12v = m12.rearrange("p (i j c) -> p i j c", i=GRP, j=2)
                for i in range(nt):
                    nc.tensor.matmul(
                        m12v[:, i, 0, :], bT[:, i, 0, :], a1hi,
                        start=True, stop=False,
                    )
                    nc.tensor.matmul(
                        m12v[:, i, 0, :], bT[0:16, i, 1, :], a1lo,
                        start=False, stop=True,
                    )
                    nc.tensor.matmul(
                        m12v[:, i, 1, :], bT[64:P, i, 1, :], a2hi,
                        start=True, stop=False,
                    )
                    nc.tensor.matmul(
                        m12v[:, i, 1, :], bT[0:80, i, 2, :], a2lo,
                        start=False, stop=True,
                    )
                st[jb]["m12"] = m12

            def S3(jb):
                t0, nt = JOBS[jb]
                m12 = st[jb].pop("m12")
                u12 = outp.tile([P, 2 * GRP * COUT], f32, tag="u12")
                nc.scalar.activation(
                    u12[:, 0 : nt * 2 * COUT], m12[:, 0 : nt * 2 * COUT], AF.Ln
                )
                st[jb]["u12"] = u12

            def S4(jb):
                t0, nt = JOBS[jb]
                u12 = st[jb].pop("u12")
                s12 = st[jb].pop("s12")
                # e = u + s (broadcast per branch/tile); u12/e12 are
                # [tile, branch, co] so each branch is a strided 3D view
                e12 = outp.tile([P, GRP, 2, COUT], f16, tag="e12")
                for j in range(2):
                    nc.gpsimd.tensor_tensor(
                        out=e12[:, 0:nt, j, :],
                        in0=u12[:, 0 : nt * 2 * COUT].rearrange(
                            "p (i j c) -> p i j c", i=nt, j=2
                        )[:, :, j, :],
                        in1=bcast(
                            s12[:, j * GRP : j * GRP + nt], [[1, nt], [0, COUT]]
                        ),
                        op=AluOpType.add,
                    )
                st[jb]["e12"] = e12

            def S5(jb):
                t0, nt = JOBS[jb]
                e12 = st.pop(jb)["e12"]
                d0 = outp.tile([P, GRP * COUT], f16, tag="d0")
                d0v = d0.rearrange("p (i c) -> p i c", i=GRP)
                nc.vector.tensor_tensor(
                    out=d0v[:, 0:nt, :],
                    in0=e12[:, 0:nt, 0, :],
                    in1=e12[:, 0:nt, 1, :],
                    op=AluOpType.max,
                )
                dg = outp.tile([P, GRP, COUT], f16, tag="dg")
                nc.vector.scalar_tensor_tensor(
                    out=dg[:, 0:nt, :].rearrange("p i c -> p (i c)"),
                    in0=d0[:, 0 : nt * COUT],
                    scalar=1.0 / T_SHARP,
                    in1=brep[:, 0 : nt * COUT],
                    op0=AluOpType.mult,
                    op1=AluOpType.add,
                )
                nc.sync.dma_start(
                    out=out_d[t0 * P : (t0 + nt) * P, :].rearrange(
                        "(t p) a -> p t a", p=P
                    ),
                    in_=dg[:, 0:nt, :],
                )

            stages = [(5, S5), (4, S4), (3, S3), (2, S2), (1, S1), (0, S0)]
            NJOBS = len(JOBS)
            for c in range(NJOBS + 5):
                for s, fn in stages:
                    jb = c - s
                    if 0 <= jb < NJOBS:
                        fn(jb)
                if c == 1:
                    # const DMAs behind jobs 0+1's patch DMAs in the HWDGE
                    # queue (their arrival gates the DVE stream start); first
                    # const consumers (S2 matmuls / S5 bias) are clocks away
                    nc.sync.dma_start(out=wpack[:, :], in_=wpack_d[:, :])
                    nc.sync.dma_start(out=brep[:, :], in_=brep_d[:, :])
    nc.compile()
    return nc


def _host_prep(inputs):
    import ml_dtypes

    x = np.asarray(inputs["x"], dtype=np.float32)
    weights = np.asarray(inputs["weights"], dtype=np.float32)
    bias = np.asarray(inputs["bias"], dtype=np.float32)
    assert x.shape == (B, C, H, W)
    assert weights.shape == (COUT, ACC)

    x_pad = np.pad(x, ((0, 0), (0, 0), (1, 1), (1, 1)), mode="edge")
    from numpy.lib.stride_tricks import sliding_window_view

    pw = sliding_window_view(x_pad, (K, K), axis=(2, 3))  # (B, C, HOUT, WOUT, K, K)
    patches = (
        np.ascontiguousarray(pw.transpose(0, 2, 3, 1, 4, 5))
        .reshape(B, NPOS, ACC)
        .astype(np.float16)
    )
    wt = weights.T.astype(np.float64)  # [144, 64]
    # packed bf16 A-matrices, partition-placed for the matmul K-chunk bases
    wpack = np.zeros((P, 4 * COUT), dtype=np.float64)
    wpack[0:P, 0:COUT] = np.exp(T_SHARP * wt[0:P])  # a1hi
    wpack[64:P, COUT : 2 * COUT] = np.exp(-T_SHARP * wt[0:64])  # a2hi
    wpack[0:80, 2 * COUT : 3 * COUT] = np.exp(-T_SHARP * wt[64:ACC])  # a2lo
    wpack[0:16, 3 * COUT : 4 * COUT] = np.exp(T_SHARP * wt[P:ACC])  # a1lo
    wpack = wpack.astype(ml_dtypes.bfloat16)
    brep = np.tile(bias.reshape(1, COUT) - DELTA, (P, GRP)).astype(np.float16)
    return patches, wpack, brep


def _get_nc():
    global _NC_CACHE
    if _NC_CACHE is None:
        _NC_CACHE = _build_bass()
    return _NC_CACHE


def _run(inputs, trace=False):
    from concourse.bass_utils import run_bass_kernel_spmd

    patches, wpack, brep = _host_prep(inputs)
    in_maps = []
    for core in range(NCORES):
        b, half = core // HALVES, core % HALVES
        shard = np.ascontiguousarray(
            patches[b, half * POS_PER_CORE : (half + 1) * POS_PER_CORE, :]
        )
        in_maps.append({"patches": shard, "wpack": wpack, "brep": brep})

    nc = _get_nc()
    res = run_bass_kernel_spmd(nc, in_maps, core_ids=list(range(NCORES)), trace=trace)

    rows_per_half = POS_PER_CORE // WOUT  # 32
    out = np.empty((B, COUT, HOUT, WOUT), dtype=np.float32)
    for core in range(NCORES):
        b, half = core // HALVES, core % HALVES
        o = res.results[core]["out"].astype(np.float32)  # [POS_PER_CORE, COUT]
        out[b, :, half * rows_per_half : (half + 1) * rows_per_half, :] = o.T.reshape(
            COUT, rows_per_half, WOUT
        )
    return out, res


def kernel(**inputs) -> np.ndarray:
    out, _ = _run(inputs, trace=_TRACE)
    return out
